# revision 74
# baseline (speedup 1.0000x reference)
"""Trainium2 Bass kernel for DigitalCapsule dynamic routing (CapsNet digit caps).

Reference math (per sample b):
    x_hat[n,o,:] = W[n,o] @ x[n,:]                       # [N=1152, O=32, Do=16], Di=8
    b = 0
    for it in range(3):
        c = softmax(b, axis=o)
        s[o,:] = sum_n c[n,o] * x_hat[n,o,:]
        v = squash(s)
        if it < 2: b += x_hat . v
    return v                                             # [O, Do]

Strategy: data-parallel over batch B=64 across 8 NeuronCores (8 samples/core).
Per core, fp16 compute / fp32 accumulate:
  - weight is pre-transformed on HOST into W-a tiles [(16n,8j) partitions,
    (o,i) free] so TensorE creates x_hat directly from a block-diagonal x
    operand (16 n's and all 8 local samples per 512-column pass).
  - x_hat lives in SBUF fp16 as [(8b,16n) partitions, (g,o,i) free].
  - s-sums run on TensorE via block-diagonal softmax-weight lhsT operands.
  - squash + V-broadcast built with two small permutation matmuls (no DMA).
  - agreements run on VectorE: fp16 2x multiply + 4-step strided add tree
    (TensorReduce has no fast mode; tree adds run at 2x).
  - per-iteration work is chunked (9 groups) to pipeline DVE agr/softmax
    against PE s-matmuls.
"""

import os
import sys

sys.path.insert(0, "/opt/trn_rl_repo")

import numpy as np
from contextlib import ExitStack

B, N, O, DO, DI = 64, 1152, 32, 16, 8
NCORES = 8
BL = B // NCORES          # 8 samples per core
G = N // 16               # 72 groups of 16 input capsules
OI = O * DO               # 512
NCH = 8                   # chunks per iteration
GPC = G // NCH            # 9 groups per chunk
EPS = 1e-7

_PROGRAM_CACHE = {}


def _build_program():
    import concourse.bass as bass
    import concourse.tile as tile
    from concourse.tile import add_dep_helper
    from concourse import bacc, mybir

    f32 = mybir.dt.float32
    f16 = mybir.dt.float16
    MULT = mybir.AluOpType.mult
    ADD = mybir.AluOpType.add
    AX = mybir.AxisListType.X
    ACT = mybir.ActivationFunctionType

    nc = bacc.Bacc("TRN2", target_bir_lowering=False, debug=False,
                   num_devices=NCORES)

    wa_d = nc.dram_tensor("wa", [128, G * OI], f16, kind="ExternalInput")
    xbd_d = nc.dram_tensor("xbd", [G, 128, 128], f16, kind="ExternalInput")
    l0_d = nc.dram_tensor("l0", [128, 128], f16, kind="ExternalInput")
    mask_d = nc.dram_tensor("mask", [128, OI], f32, kind="ExternalInput")
    lhsmask_d = nc.dram_tensor("lhsmask", [128, G * 128], f16,
                               kind="ExternalInput")
    perm_d = nc.dram_tensor("perm", [128, 128], f16, kind="ExternalInput")
    lsum_d = nc.dram_tensor("lsum", [128, 128], f16, kind="ExternalInput")
    s2_d = nc.dram_tensor("s2", [128, OI], f32, kind="ExternalOutput")

    with tile.TileContext(nc) as tc, ExitStack() as ctx:
        pers = ctx.enter_context(tc.tile_pool(name="pers", bufs=1))
        xh = pers.tile([128, G * OI], f16)          # 9.2 MB
        l0 = pers.tile([128, 128], f16)
        mask = pers.tile([128, OI], f32)
        perm = pers.tile([128, 128], f16)
        lsum = pers.tile([128, 128], f16)
        warm = pers.tile([128, 2], f32)

        ps_s = ctx.enter_context(tc.tile_pool(name="ps_s", bufs=2, space="PSUM"))
        s0a = ps_s.tile([128, 256], f32, tag="sa")
        s0b = ps_s.tile([128, 256], f32, tag="sb")

        # ---------------- stage 1: x_hat create + s0 -----------------------
        with tc.tile_pool(name="wa", bufs=3) as wa_p, \
             tc.tile_pool(name="ps_c", bufs=2, space="PSUM") as ps_c, \
             tc.tile_pool(name="xbd", bufs=1) as xbd_p:

            xbd = xbd_p.tile([128, G * 128], f16)

            for t in range(9):
                # per-tile xbd slice so the first matmuls start early
                nc.sync.dma_start(
                    xbd[:, t * 8 * 128:(t + 1) * 8 * 128].rearrange(
                        "p (g m) -> p g m", g=8),
                    xbd_d.ap()[t * 8:(t + 1) * 8].rearrange("g p m -> p g m"))
                if t == 0:
                    nc.sync.dma_start(l0[:], l0_d.ap())
                wa = wa_p.tile([128, 8 * OI], f16)
                for pr4 in range(4):
                    sl = slice(pr4 * 2 * OI, (pr4 + 1) * 2 * OI)
                    # alternate between the two HWDGE queues (SP / ACT) so
                    # the weight stream uses both DMA pipes
                    eng = nc.sync if pr4 % 2 == 0 else nc.scalar
                    eng.dma_start(
                        wa[:, sl],
                        wa_d.ap()[:, t * 8 * OI:(t + 1) * 8 * OI][:, sl])
                for pr in range(4):
                    pc = ps_c.tile([128, 1024], f32)
                    for half in range(2):
                        g = t * 8 + pr * 2 + half
                        nc.tensor.matmul(pc[:, half * 512:(half + 1) * 512],
                                         xbd[:, g * 128:(g + 1) * 128],
                                         wa[:, (pr * 2 + half) * OI:
                                            (pr * 2 + half + 1) * OI],
                                         start=True, stop=True)
                    g0 = t * 8 + pr * 2
                    dst = xh[:, g0 * OI:(g0 + 2) * OI]
                    # strict ACT/DVE alternation: each engine's copy stream
                    # (~1.0/1.3us per pair) stays ahead of PE's 1.7us/pair,
                    # so PSUM slots recycle without stalling the matmuls
                    if (t * 4 + pr) % 2 == 0:
                        nc.vector.tensor_copy(dst, pc[:])
                    else:
                        nc.scalar.copy(dst, pc[:])
                if t == 0:
                    # pin the exp_and_others ACT table now (exp + copy share
                    # a set; squash uses DVE-only rsqrt): the 1.3us table
                    # load lands mid-stage-1 instead of on the squash0 path
                    nc.scalar.activation(warm[:], l0[:, 0:2], ACT.Exp)
                # s0 accumulation (uniform c = 1/32), one tile behind the
                # x_hat matmuls so PE never waits on the PSUM->SBUF copies
                tlist = [t - 1] if t > 0 else []
                if t == 8:
                    tlist.append(8)
                for ts0 in tlist:
                    # a-half first: its stop lands ~1.7us before b's, letting
                    # squash0's half-0 chain overlap the b-half matmuls
                    for gs in range(8):
                        g = ts0 * 8 + gs
                        nc.tensor.matmul(s0a[:], l0[:],
                                         xh[:, g * OI:g * OI + 256],
                                         start=(g == 0), stop=(g == G - 1),
                                         skip_group_check=True)
                    for gs in range(8):
                        g = ts0 * 8 + gs
                        nc.tensor.matmul(s0b[:], l0[:],
                                         xh[:, g * OI + 256:(g + 1) * OI],
                                         start=(g == 0), stop=(g == G - 1),
                                         skip_group_check=True)

        # constants are only needed from the first squash on; DMA them after
        # stage 1's weight traffic is queued
        nc.sync.dma_start(mask[:], mask_d.ap())
        nc.sync.dma_start(perm[:], perm_d.ap())
        nc.sync.dma_start(lsum[:], lsum_d.ap())

        # ---------------- stage 2: routing iterations ----------------------
        with tc.tile_pool(name="it", bufs=1) as it_p, \
             tc.tile_pool(name="tmp", bufs=1) as tmp_p, \
             tc.tile_pool(name="sq", bufs=1) as sq_p, \
             tc.tile_pool(name="ps_x", bufs=2, space="PSUM") as ps_x:

            bstate = it_p.tile([128, G * O], f16)
            bdel = it_p.tile([128, G * O], f16)
            ex = it_p.tile([128, G * O], f16)
            zr = it_p.tile([128, G], f32)
            rzr = it_p.tile([128, G], f32)
            cvals = it_p.tile([128, G * O], f16)
            lhsA = it_p.tile([128, G * 128], f16)
            lhsB = it_p.tile([128, G * 128], f16)
            lhsmask = it_p.tile([128, G * 128], f16)
            nc.sync.dma_start(lhsmask[:], lhsmask_d.ap())
            V = it_p.tile([128, OI], f16)
            s2sb = it_p.tile([128, OI], f32)

            # separate agr scratch for the GpSimd chunk so it overlaps DVE
            tmpt, t1, t2, t3 = [], [], [], []
            for e in range(2):
                for lst, sz in ((tmpt, OI), (t1, 256), (t2, 128), (t3, 64)):
                    tl = tmp_p.tile([128, GPC * sz], f16, tag=f"agr{sz}_{e}",
                                    name=f"agr{sz}_{e}")
                    lst.append(tl)

            sperm = sq_p.tile([128, OI], f16)
            sm = sq_p.tile([128, OI], f32)
            prodj = sq_p.tile([128, OI], f32)
            vm = sq_p.tile([128, OI], f16)
            n2 = sq_p.tile([128, 2], f32)
            n2e = sq_p.tile([128, 2], f32)
            t0 = sq_p.tile([128, 2], f32)
            q0 = sq_p.tile([128, 2], f32)
            rt = sq_p.tile([128, 2], f32)
            a1 = sq_p.tile([128, 2], f32)
            gf = sq_p.tile([128, 2], f32)

            def squash_to_V(psA, psB):
                # s lives as [p=(8b,16o_l), (h, o', i)]; permute partitions to
                # (16o_l, 8b), mask the o'==o_l diagonal, squash per capsule,
                # then V[(b,nl), (o,i)] = sum_ol of masked rows (b-matched).
                # per-half chain so ACT copy / PE permute / DVE mask+accum
                # of half 1 overlap half 0's downstream ops
                BYP = mybir.AluOpType.bypass
                ps_p = ps_x.tile([128, OI], f32, tag="px")
                for h, ps in ((0, psA), (1, psB)):
                    sl = slice(h * 256, (h + 1) * 256)
                    nc.scalar.copy(sperm[:, sl], ps[:])
                    nc.tensor.matmul(ps_p[:, sl], perm[:], sperm[:, sl],
                                     start=True, stop=True)
                    nc.vector.tensor_tensor(sm[:, sl], ps_p[:, sl],
                                            mask[:, sl], op=MULT)
                    # n2[:, h] = sum(sm_h^2) fused: (ps_perm bypass) * sm
                    # (mask is 0/1 so ps_perm * sm == sm^2 where it counts)
                    nc.vector.scalar_tensor_tensor(
                        prodj[:, sl], ps_p[:, sl], 0.0, sm[:, sl],
                        op0=BYP, op1=MULT, accum_out=n2[:, h:h + 1])
                # g = n2 * rsqrt(n2 + eps) / (1 + n2); feeds routing only.
                # rsqrt via magic-constant bit trick, all on DVE
                # (ACT sqrt would force a function-table swap ~1.3us).
                I32 = mybir.dt.int32
                SHR = mybir.AluOpType.logical_shift_right
                nc.vector.tensor_scalar_add(n2e[:], n2[:], EPS)
                nc.vector.tensor_scalar(t0[:].bitcast(I32),
                                        n2e[:].bitcast(I32), 1, None, op0=SHR)
                nc.vector.tensor_scalar(t0[:].bitcast(I32),
                                        t0[:].bitcast(I32),
                                        -1, 0x5f3759df, op0=MULT, op1=ADD)
                # no Newton refinement: the ~3% rsqrt error only perturbs
                # routing weights (verified ~5e-4 end-to-end)
                nc.vector.tensor_scalar_add(a1[:], n2[:], 1.0)
                nc.vector.reciprocal(rt[:], a1[:])
                nc.vector.tensor_tensor(gf[:], t0[:], rt[:], op=MULT)
                nc.vector.tensor_tensor(gf[:], gf[:], n2[:], op=MULT)
                for h in (0, 1):
                    sl = slice(h * 256, (h + 1) * 256)
                    nc.vector.tensor_scalar_mul(vm[:, sl], sm[:, sl],
                                                gf[:, h:h + 1])
                ps_v = ps_x.tile([128, OI], f32, tag="px")
                nc.tensor.matmul(ps_v[:], lsum[:], vm[:],
                                 start=True, stop=True)
                nc.scalar.copy(V[:], ps_v[:])

            chain = {"exp": None, "cvals": None}

            def iteration(first, psA, psB):
                """agr vs V -> b update -> softmax -> lhs -> s matmuls,
                pipelined over NCH chunks of GPC groups."""
                def emit_agr(ve, ch, e):
                    g0 = ch * GPC
                    csl = slice(g0 * O, (g0 + GPC) * O)
                    ve.tensor_tensor(
                        tmpt[e][:].rearrange("p (q x) -> p q x", q=GPC),
                        xh[:, g0 * OI:(g0 + GPC) * OI].rearrange(
                            "p (q x) -> p q x", q=GPC),
                        V[:].unsqueeze(1).broadcast_to([128, GPC, OI]),
                        op=MULT)
                    ve.tensor_tensor(
                        t1[e][:].rearrange("p (s i) -> p s i", i=8),
                        tmpt[e][:].rearrange("p (s i) -> p s i", i=16)[:, :, :8],
                        tmpt[e][:].rearrange("p (s i) -> p s i", i=16)[:, :, 8:],
                        op=ADD)
                    ve.tensor_tensor(
                        t2[e][:].rearrange("p (s i) -> p s i", i=4),
                        t1[e][:].rearrange("p (s i) -> p s i", i=8)[:, :, :4],
                        t1[e][:].rearrange("p (s i) -> p s i", i=8)[:, :, 4:],
                        op=ADD)
                    ve.tensor_tensor(
                        t3[e][:].rearrange("p (s i) -> p s i", i=2),
                        t2[e][:].rearrange("p (s i) -> p s i", i=4)[:, :, :2],
                        t2[e][:].rearrange("p (s i) -> p s i", i=4)[:, :, 2:],
                        op=ADD)
                    bdst = bstate if first else bdel
                    ve.tensor_tensor(
                        bdst[:, csl].rearrange("p (s u) -> p s u", u=1),
                        t3[e][:].rearrange("p (s i) -> p s i", i=2)[:, :, 0:1],
                        t3[e][:].rearrange("p (s i) -> p s i", i=2)[:, :, 1:2],
                        op=ADD)
                    if not first:
                        ve.tensor_add(bstate[:, csl], bstate[:, csl],
                                      bdel[:, csl])

                def emit_rest(ch, last=False):
                    g0 = ch * GPC
                    csl = slice(g0 * O, (g0 + GPC) * O)
                    # --- softmax over o within the chunk ---
                    e_inst = nc.scalar.activation(ex[:, csl], bstate[:, csl],
                                                  ACT.Exp)
                    # order-only chain: the scheduler's readiness model
                    # underestimates GpSimd, so without this it puts the
                    # Pool-fed chunk's exp FIRST on ACT and head-of-line
                    # blocks every other chunk's softmax
                    if chain["exp"] is not None:
                        add_dep_helper(e_inst.ins, chain["exp"].ins,
                                       sync=False, reason="exp order")
                    chain["exp"] = e_inst
                    r_inst = nc.vector.tensor_reduce(
                        zr[:, g0:g0 + GPC],
                        ex[:, csl].rearrange("p (g o) -> p g o", g=GPC),
                        axis=AX, op=ADD)
                    if chain["cvals"] is not None:
                        add_dep_helper(r_inst.ins, chain["cvals"].ins,
                                       sync=False, reason="dve order")
                    nc.vector.reciprocal(rzr[:, g0:g0 + GPC],
                                         zr[:, g0:g0 + GPC])
                    cv_eng = nc.gpsimd if ch in (3, 4, 5) else nc.vector
                    cv_inst = cv_eng.tensor_tensor(
                        cvals[:, csl].rearrange("p (g o) -> p g o", g=GPC),
                        ex[:, csl].rearrange("p (g o) -> p g o", g=GPC),
                        rzr[:, g0:g0 + GPC].unsqueeze(2).broadcast_to(
                            [128, GPC, O]),
                        op=MULT)
                    if cv_eng is nc.vector:
                        # DVE-queue order anchor (don't chain across engines)
                        chain["cvals"] = cv_inst
                    # --- lhs build: c expanded to b-matched blocks ---
                    msl = slice(g0 * 128, (g0 + GPC) * 128)
                    for h, lhs in ((0, lhsA), (1, lhsB)):
                        csrc = cvals[:, csl].rearrange(
                            "p (g o) -> p g o", g=GPC)[
                            :, :, h * 16:(h + 1) * 16].unsqueeze(2).broadcast_to(
                            [128, GPC, 8, 16])
                        # early chunks on DVE (Pool is busy with its agr),
                        # late chunks on Pool, last chunk on DVE (short tail)
                        if last or ch < 2 or (ch == 2 and h == 0):
                            eng = nc.vector
                        else:
                            eng = nc.gpsimd
                        eng.tensor_tensor(
                            lhs[:, msl].rearrange("p (g b o) -> p g b o",
                                                  g=GPC, b=8),
                            csrc,
                            lhsmask[:, msl].rearrange("p (g b o) -> p g b o",
                                                      g=GPC, b=8),
                            op=MULT)
                    # --- s matmuls for this chunk ---
                    # start/stop follow EMISSION order (ch0 first, `last` last)
                    for q in range(GPC):
                        g = g0 + q
                        nc.tensor.matmul(psA[:], lhsA[:, g * 128:(g + 1) * 128],
                                         xh[:, g * OI:g * OI + 256],
                                         start=(g == 0),
                                         stop=(last and q == GPC - 1),
                                         skip_group_check=True)
                    for q in range(GPC):
                        g = g0 + q
                        nc.tensor.matmul(psB[:], lhsB[:, g * 128:(g + 1) * 128],
                                         xh[:, g * OI + 256:(g + 1) * OI],
                                         start=(g == 0),
                                         stop=(last and q == GPC - 1),
                                         skip_group_check=True)

                # chunk 7's agreement runs on GpSimd, emitted FIRST so it
                # overlaps the DVE chunks; its softmax/lhs/matmuls slot in
                # after DVE chunk 3 (GpSimd agr done by then; the exp/reduce
                # order chains stop the scheduler from head-of-line blocking
                # ACT/DVE on the Pool result). ch6 is emitted last -> short
                # all-DVE tail before the stop-flagged matmuls.
                emit_agr(nc.gpsimd, NCH - 1, 0)
                for ch in range(NCH - 1):
                    emit_agr(nc.vector, ch, 1)
                    emit_rest(ch, last=(ch == NCH - 2))
                    if ch == 3:
                        emit_rest(NCH - 1)

            # ---- iteration 0 (uniform c handled by s0 in stage 1)
            squash_to_V(s0a, s0b)
            s1a = ps_s.tile([128, 256], f32, tag="sa")
            s1b = ps_s.tile([128, 256], f32, tag="sb")
            iteration(True, s1a, s1b)

            # ---- iteration 1
            squash_to_V(s1a, s1b)
            s2a = ps_s.tile([128, 256], f32, tag="sa")
            s2b = ps_s.tile([128, 256], f32, tag="sb")
            iteration(False, s2a, s2b)

            # ---- iteration 2: ship raw s2 (host extracts + squashes);
            # halves on separate engines + DMA queues so copy/transfer overlap
            nc.scalar.copy(s2sb[:, :256], s2a[:])
            nc.sync.dma_start(s2_d.ap()[:, :256], s2sb[:, :256])
            nc.vector.tensor_copy(s2sb[:, 256:], s2b[:])
            nc.scalar.dma_start(s2_d.ap()[:, 256:], s2sb[:, 256:])

    nc.compile()
    return nc


def _host_prep(x_shard):
    """Block-diagonal x operand: xbd[g, nl*8+j, b*16+nl] = x[b, g*16+nl, j]."""
    xr = x_shard.reshape(BL, G, 16, DI).transpose(1, 2, 3, 0)  # [g, nl, j, b]
    xbd = np.zeros((G, 128, 128), np.float16)
    for nl in range(16):
        xbd[:, nl * 8:(nl + 1) * 8, nl::16] = xr[:, nl].astype(np.float16)
    return xbd


def _host_wa(weight):
    """W-a layout: wa[nl*8+j, g*512 + o*16+i] = weight[g*16+nl, o, i, j]."""
    w = weight.reshape(G, 16, O, DO, DI)            # [g, nl, o, i, j]
    w = w.transpose(1, 4, 0, 2, 3)                  # [nl, j, g, o, i]
    return np.ascontiguousarray(w.reshape(128, G * OI).astype(np.float16))


def _host_static():
    # s-matmul lhsT M-order (8b,16o): col m = b*16 + o_local
    # l0[(b,n)-row, (b',o)-col] = 1/32 iff b == b'
    l0 = np.zeros((8, 16, 8, 16), np.float16)
    for b in range(8):
        l0[b, :, b, :] = np.float16(1.0 / 32.0)
    # mask for the PERMUTED s layout [p=(ol,b), col=(h,o',i)]: 1 iff o' == ol
    mask = np.zeros((16, 8, 2, 16, 16), np.float32)
    for ol in range(16):
        mask[ol, :, :, ol, :] = 1.0
    # lhsmask[(b,n)-row, (g, b', o)] = 1 iff b == b'
    lm = np.zeros((8, 16, G, 8, 16), np.float16)
    for b in range(8):
        lm[b, :, :, b, :] = 1.0
    # perm[(b,o)-row, (o',b')-col] = 1 iff b==b' and o==o'
    perm = np.zeros((8, 16, 16, 8), np.float16)
    for b in range(8):
        for o in range(16):
            perm[b, o, o, b] = 1.0
    # lsum[(ol,b)-row, (b',nl)-col] = 1 iff b==b'
    ls = np.zeros((16, 8, 8, 16), np.float16)
    for b in range(8):
        ls[:, b, b, :] = 1.0
    return (l0.reshape(128, 128), mask.reshape(128, OI),
            lm.reshape(128, G * 128), perm.reshape(128, 128),
            ls.reshape(128, 128))


def _extract_squash(s2raw):
    """s2raw [128, 512] -> v2 [BL, O, DO] (diag extract + squash, fp32)."""
    s = np.zeros((BL, O, DO), np.float64)
    r = s2raw.reshape(8, 16, 2, 16, 16).astype(np.float64)  # [b, ol, h, o', i]
    for ol in range(16):
        for h in range(2):
            s[:, h * 16 + ol, :] = r[:, ol, h, ol, :]
    n2 = np.sum(s * s, axis=-1, keepdims=True)
    v = (n2 / (1.0 + n2) / np.sqrt(n2 + EPS)) * s
    return v.astype(np.float32)


def kernel(x, weight):
    from concourse.bass_utils import run_bass_kernel_spmd

    x = np.asarray(x, dtype=np.float32)
    weight = np.asarray(weight, dtype=np.float32)

    if "nc" not in _PROGRAM_CACHE:
        _PROGRAM_CACHE["nc"] = _build_program()
    nc = _PROGRAM_CACHE["nc"]

    l0, mask, lhsmask, perm, lsum = _host_static()
    wa = _host_wa(weight)
    in_maps = []
    for c in range(NCORES):
        xbd = _host_prep(x[c * BL:(c + 1) * BL])
        in_maps.append({"wa": wa, "xbd": xbd, "l0": l0, "mask": mask,
                        "lhsmask": lhsmask, "perm": perm, "lsum": lsum})

    res = run_bass_kernel_spmd(nc, in_maps, core_ids=list(range(NCORES)),
                               trace=bool(int(os.environ.get("KERNEL_TRACE", "0"))))
    _PROGRAM_CACHE["last_results"] = res

    out = np.empty((B, O, DO), np.float32)
    for c in range(NCORES):
        out[c * BL:(c + 1) * BL] = _extract_squash(res.results[c]["s2"])
    return out
